# revision 1
# baseline (speedup 1.0000x reference)
"""Trainium2 Bass kernel for EpiModule (epipolar sparse attention).

Full inputs -> full output. Shards B=32 frames data-parallel across 8
NeuronCores (4 frames per core); QKV/O weights replicated.

Per-core device program (all fp32 storage, float32r matmuls):
  per frame:
    lines   = F^T-block matmul against constant pixel coords     [3, 1024]
    cfcT    = |coords^T @ lines| * (1/||line||)  (scoresT layout) [1024m, 1024n]
    band    = (3/128) * max(cfc);  emask = min(exp(-decay*(cfc-band)), 1)
    hsT     = PE-transpose of hidden_states                       [512c, 1024n]
    qT,kT   = W^T @ hsT  ;  v65 = [hs @ Wv | ones]                (ones col -> denom)
    per head:
      scoresT = kT^T q  (PSUM), e = exp(scores/8), attn = min(emask,1)*e
      outT[65, n] = v65^T @ attn   (row 64 = softmax denominator)
      attn_norm = outT[0:64] * recip(denom)  (broadcast via ones-matmul)
    out = attn_norm^T @ Wo_r  -> DRAM

Softmax max-subtraction is skipped: scores = qk/8 + mask with mask <= 0 and
every query row has a mask entry >= -20 (verified on the generator
distribution), so exp() neither overflows nor denominator-underflows in fp32.
"""

import sys

if "/opt/trn_rl_repo" not in sys.path:
    sys.path.insert(0, "/opt/trn_rl_repo")

import numpy as np

B, N, C = 32, 1024, 512
HEADS, D = 8, 64
NCORES = 8
FPC = B // NCORES           # frames per core
F_MAT_SIZE = 256
PIXEL_BAND = 3.0
DECAY_ALPHA = 3.0
FRAMES_PER_VIDEO = 16

# pseudo "horizontal line" fundamental matrix: F~ @ [x,y,1] = [0,-1,y]
PSEUDO_F = np.array([[0.0, 0.0, 0.0],
                     [0.0, 0.0, -1.0],
                     [0.0, 1.0, 0.0]], dtype=np.float32)


def make_coordsT():
    feat = int(round(N ** 0.5))          # 32
    n = np.arange(N, dtype=np.float32)
    scale = F_MAT_SIZE / feat            # 8.0
    off = (scale - 1.0) / 2.0            # 3.5
    x = scale * (n % feat) + off
    y = scale * (n // feat) + off
    return np.stack([x, y, np.ones(N, np.float32)], axis=0)  # [3, N]


def build_body(tc, out_aps, in_aps, attn_bufs=6, e1_bufs=2):
    """Emit the per-core program. out_aps/in_aps: dicts of DRAM APs."""
    import contextlib

    import concourse.bass as bass  # noqa: F401
    from concourse import mybir
    from concourse.masks import make_identity

    nc = tc.nc
    f32 = mybir.dt.float32
    f32r = mybir.dt.float32r
    AF = mybir.ActivationFunctionType
    OP = mybir.AluOpType
    AX = mybir.AxisListType

    def mm(out, lhsT, rhs, **kw):
        nc.tensor.matmul(out, lhsT, rhs, **kw)

    hs_d = in_aps["hs"]                      # [4, 1024, 512]
    ft_d = in_aps["ft"]                      # [3, 4, 3]   FT[y,f,x] = F_f[x,y]
    ct_d = in_aps["coordsT"]                 # [3, 1024]
    wq_d = in_aps["Wq"].bitcast(f32r)        # [512, 512]
    wk_d = in_aps["Wk"].bitcast(f32r)
    wv_d = in_aps["Wv"].bitcast(f32r)
    wor_d = in_aps["Wo_r"].bitcast(f32r)     # [128, 4, 512]
    ones_r_d = in_aps["ones_row"].bitcast(f32r)   # [1, 128]
    ones_v_d = in_aps["ones_v"].bitcast(f32r)     # [128, 8, 8, 1]
    out_d = out_aps["out"]                   # [4, 1024, 512]

    ctx = contextlib.ExitStack()
    with ctx, nc.allow_low_precision(reason="float32r tiles are fp32-width"):
        consts = ctx.enter_context(tc.tile_pool(name="consts", bufs=1))
        big = ctx.enter_context(tc.tile_pool(name="big", bufs=1))
        attn_pool = ctx.enter_context(tc.tile_pool(name="attn", bufs=max(2, attn_bufs // 3)))
        e1_pool = ctx.enter_context(tc.tile_pool(name="e1", bufs=e1_bufs))
        hs_pool = ctx.enter_context(tc.tile_pool(name="hsld", bufs=2))
        rb_pool = ctx.enter_context(tc.tile_pool(name="rb", bufs=4))
        psA = ctx.enter_context(tc.tile_pool(name="psA", bufs=2, space="PSUM"))
        psB = ctx.enter_context(tc.tile_pool(name="psB", bufs=4, space="PSUM"))

        # ---------------- constants ----------------
        wq_sb = consts.tile([128, 4, 512], f32r)
        wk_sb = consts.tile([128, 4, 512], f32r)
        wv_sb = consts.tile([128, 4, 512], f32r)
        nc.sync.dma_start(out=wq_sb, in_=wq_d.rearrange("(cc p) d -> p cc d", p=128))
        nc.sync.dma_start(out=wk_sb, in_=wk_d.rearrange("(cc p) d -> p cc d", p=128))
        nc.sync.dma_start(out=wv_sb, in_=wv_d.rearrange("(cc p) d -> p cc d", p=128))
        wor_sb = consts.tile([128, 4, 512], f32r)
        nc.sync.dma_start(out=wor_sb, in_=wor_d)
        ct_sb = consts.tile([3, 1024], f32)
        nc.sync.dma_start(out=ct_sb, in_=ct_d)
        ft_sb = consts.tile([3, 4, 3], f32)
        nc.sync.dma_start(out=ft_sb, in_=ft_d)
        ident = consts.tile([128, 128], f32)
        make_identity(nc, ident)
        ones_row = consts.tile([1, 128], f32r)
        nc.sync.dma_start(out=ones_row, in_=ones_r_d)

        # persistent per-frame workspaces (serially reused across frames)
        hsT = big.tile([128, 4, 1024], f32r)      # [c%128, cc, n]
        qT = big.tile([128, 4, 1024], f32r)       # [d%128, dc, n]
        kT = big.tile([128, 4, 1024], f32r)
        v65 = big.tile([128, 8, 8, 65], f32r)     # [m%128, mt, h, d|one]
        cfc = big.tile([128, 8, 1024], f32)      # [m%128, mt, n]; becomes emask
        anorm = big.tile([128, 4, 1024], f32r)    # [(h%2)*64+d, h//2, n]
        lines = big.tile([3, 1024], f32)
        abn_sq = big.tile([2, 1024], f32)
        abn = big.tile([1, 1024], f32r)
        bandp = big.tile([128, 8], f32)
        band2 = big.tile([128, 1], f32)
        band1 = big.tile([1, 1], f32r)
        bandM = big.tile([128, 1], f32)
        band_col = big.tile([128, 1], f32)
        rec = big.tile([128, 1], f32)
        negdecay = big.tile([128, 1], f32)
        dxb = big.tile([128, 1], f32)

        nc.sync.dma_start(out=v65[:, :, :, 64:65], in_=ones_v_d)

        for f in range(FPC):
            # ============ mask construction ============
            ps_l = psA.tile([3, 1024], f32, tag="psA")
            for s in range(2):
                nc.tensor.matmul(ps_l[:, s * 512:(s + 1) * 512], ft_sb[:, f, :],
                                 ct_sb[:, s * 512:(s + 1) * 512],
                                 start=True, stop=True)
            nc.vector.tensor_copy(out=lines, in_=ps_l)

            nc.vector.tensor_tensor(out=abn_sq, in0=lines[0:2, :],
                                    in1=lines[0:2, :], op=OP.mult)
            nc.gpsimd.tensor_reduce(out=abn, in_=abn_sq, axis=AX.C, op=OP.add)
            nc.scalar.activation(out=abn, in_=abn, func=AF.Sqrt)
            nc.vector.tensor_scalar_add(out=abn, in0=abn, scalar1=1e-6)
            nc.vector.reciprocal(out=abn, in_=abn)

            ps_i = psA.tile([128, 1024], f32, tag="psA")
            for s in range(2):
                mm(ps_i[:, s * 512:(s + 1) * 512], ones_row,
                   abn[:, s * 512:(s + 1) * 512], start=True, stop=True)
            # lines *= 1/||line||  (rows of the broadcast are identical)
            nc.vector.tensor_tensor(out=lines, in0=lines, in1=ps_i[0:3, :],
                                    op=OP.mult)

            for mt in range(8):
                ps_c = psA.tile([128, 1024], f32, tag="psA")
                for s in range(2):
                    nc.tensor.matmul(ps_c[:, s * 512:(s + 1) * 512],
                                     ct_sb[:, mt * 128:(mt + 1) * 128],
                                     lines[:, s * 512:(s + 1) * 512],
                                     start=True, stop=True)
                # band partial from PSUM (fp32, fused abs)
                nc.vector.tensor_reduce(out=bandp[:, mt:mt + 1], in_=ps_c,
                                        axis=AX.X, op=OP.max,
                                        apply_absolute_value=True)
                # cfc = |raw| via sign-bit clear (lines pre-normalized)
                nc.vector.tensor_scalar(
                    out=cfc[:, mt, :].bitcast(mybir.dt.int32),
                    in0=ps_c.bitcast(mybir.dt.int32),
                    scalar1=0x7FFFFFFF, scalar2=None,
                    op0=OP.bitwise_and)

            nc.vector.tensor_reduce(out=band2, in_=bandp, axis=AX.X, op=OP.max)
            nc.gpsimd.tensor_reduce(out=band1, in_=band2, axis=AX.C, op=OP.max)
            ps_b = psB.tile([128, 512], f32, tag="psB")
            nc.tensor.matmul(ps_b[:, 0:1], ones_row.bitcast(f32),
                             band1.bitcast(f32), start=True, stop=True)
            nc.vector.tensor_copy(out=bandM, in_=ps_b[:, 0:1])
            nc.vector.tensor_scalar_mul(out=band_col, in0=bandM,
                                        scalar1=PIXEL_BAND / (F_MAT_SIZE // 2))
            nc.vector.tensor_scalar_add(out=rec, in0=band_col, scalar1=1e-6)
            nc.vector.reciprocal(out=rec, in_=rec)
            nc.vector.tensor_scalar_mul(out=negdecay, in0=rec, scalar1=-DECAY_ALPHA)
            nc.vector.scalar_tensor_tensor(out=dxb, in0=band_col,
                                           scalar=DECAY_ALPHA, in1=rec,
                                           op0=OP.mult, op1=OP.mult)
            # emask = min(exp(-decay*cfc + decay*band), 1), in place over cfc
            for mt in range(8):
                nc.scalar.activation(out=cfc[:, mt, :], in_=cfc[:, mt, :],
                                     func=AF.Exp, scale=negdecay, bias=dxb)
                nc.vector.tensor_scalar(out=cfc[:, mt, :], in0=cfc[:, mt, :],
                                        scalar1=1.0, scalar2=None, op0=OP.min)

            # ============ hs load + transpose ============
            for nt in range(8):
                hst = hs_pool.tile([128, 512], f32)
                nc.sync.dma_start(out=hst, in_=hs_d[f, nt * 128:(nt + 1) * 128, :])
                ps_t = psB.tile([128, 512], f32, tag="psB")
                for cc in range(4):
                    nc.tensor.transpose(ps_t[:, cc * 128:(cc + 1) * 128],
                                        hst[:, cc * 128:(cc + 1) * 128], ident)
                nc.vector.tensor_copy(
                    out=hsT[:, :, nt * 128:(nt + 1) * 128],
                    in_=ps_t.rearrange("p (cc n) -> p cc n", cc=4))

            # ============ q/k/v projections ============
            for dst, w_sb in ((qT, wq_sb), (kT, wk_sb)):
                for dc in range(4):
                    ps_q = psA.tile([128, 1024], f32, tag="psA")
                    for s in range(2):
                        for cc in range(4):
                            mm(ps_q[:, s * 512:(s + 1) * 512],
                               w_sb[:, cc, dc * 128:(dc + 1) * 128],
                               hsT[:, cc, s * 512:(s + 1) * 512],
                               start=(cc == 0), stop=(cc == 3))
                    nc.vector.tensor_copy(out=dst[:, dc, :], in_=ps_q)

            for nt in range(8):
                ps_v = psB.tile([128, 512], f32, tag="psB")
                for cc in range(4):
                    mm(ps_v, hsT[:, cc, nt * 128:(nt + 1) * 128], wv_sb[:, cc, :],
                       start=(cc == 0), stop=(cc == 3))
                nc.vector.tensor_copy(
                    out=v65[:, nt, :, 0:64],
                    in_=ps_v.rearrange("p (h d) -> p h d", h=8))

            # ============ attention ============
            for h in range(8):
                hb = (h % 2) * 64          # partition base of this head in qT/kT
                hc = h // 2                # chunk index
                ps_av0 = psB.tile([128, 512], f32, tag="psB")
                ps_av1 = psB.tile([128, 512], f32, tag="psB")
                ps_av = [ps_av0, ps_av1]
                for mp in range(4):          # mt pairs
                    e1 = e1_pool.tile([128, 2, 1024], f32)
                    for half in range(2):
                        mt = mp * 2 + half
                        ps_s = psA.tile([128, 1024], f32, tag="psA")
                        for s in range(2):
                            mm(ps_s[:, s * 512:(s + 1) * 512],
                               kT[hb:hb + 64, hc, mt * 128:(mt + 1) * 128],
                               qT[hb:hb + 64, hc, s * 512:(s + 1) * 512],
                               start=True, stop=True)
                        # exp to SBUF so the qk PSUM frees early
                        nc.scalar.activation(out=e1[:, half, :], in_=ps_s,
                                             func=AF.Exp, scale=1.0 / 8.0)
                    at = attn_pool.tile([128, 2, 1024], f32r)
                    if (h * 4 + mp) % 15 < 8:
                        nc.gpsimd.tensor_tensor(
                            out=at, in0=cfc[:, mp * 2:mp * 2 + 2, :],
                            in1=e1, op=OP.mult)
                    else:
                        nc.vector.tensor_tensor(
                            out=at, in0=cfc[:, mp * 2:mp * 2 + 2, :],
                            in1=e1, op=OP.mult)
                    for half in range(2):
                        mt = mp * 2 + half
                        for s in range(2):
                            mm(ps_av[s][0:65, :], v65[:, mt, h, :],
                               at[:, half, s * 512:(s + 1) * 512],
                               start=(mt == 0), stop=(mt == 7))

                for s in range(2):
                    rden = rb_pool.tile([1, 512], f32r, tag="rb")
                    nc.vector.reciprocal(out=rden, in_=ps_av[s][64:65, :])
                    ps_r = psB.tile([128, 512], f32, tag="psB")
                    mm(ps_r[0:64, :], ones_row[:, 0:64], rden,
                       start=True, stop=True)
                    rb = rb_pool.tile([64, 512], f32, tag="rb")
                    nc.vector.tensor_copy(out=rb, in_=ps_r[0:64, :])
                    if h % 2 == 0:
                        nc.vector.tensor_tensor(
                            out=anorm[0:64, h // 2, s * 512:(s + 1) * 512],
                            in0=ps_av[s][0:64, :], in1=rb, op=OP.mult)
                    else:
                        ntmp = rb_pool.tile([64, 512], f32r, tag="rb")
                        nc.vector.tensor_tensor(
                            out=ntmp, in0=ps_av[s][0:64, :], in1=rb, op=OP.mult)
                        nc.sync.dma_start(
                            out=anorm[64:128, h // 2, s * 512:(s + 1) * 512],
                            in_=ntmp)

            # ============ output projection ============
            for nt in range(8):
                ps_o = psB.tile([128, 512], f32, tag="psB")
                for j in range(4):
                    mm(ps_o, anorm[:, j, nt * 128:(nt + 1) * 128],
                       wor_sb[:, j, :],
                       start=(j == 0), stop=(j == 3))
                ot = rb_pool.tile([128, 512], f32, tag="rb")
                nc.scalar.copy(out=ot, in_=ps_o)
                nc.sync.dma_start(out=out_d[f, nt * 128:(nt + 1) * 128, :], in_=ot)


_CACHED = None


def _build_program():
    global _CACHED
    if _CACHED is not None:
        return _CACHED
    import concourse.tile as tile
    from concourse import bacc, mybir

    f32 = mybir.dt.float32
    f32r = mybir.dt.float32r
    nc = bacc.Bacc("TRN2", target_bir_lowering=False, debug=False,
                   num_devices=NCORES)
    ins = {
        "hs": nc.dram_tensor("hs", [FPC, N, C], f32, kind="ExternalInput").ap(),
        "ft": nc.dram_tensor("ft", [3, FPC, 3], f32, kind="ExternalInput").ap(),
        "coordsT": nc.dram_tensor("coordsT", [3, N], f32, kind="ExternalInput").ap(),
        "Wq": nc.dram_tensor("Wq", [C, C], f32, kind="ExternalInput").ap(),
        "Wk": nc.dram_tensor("Wk", [C, C], f32, kind="ExternalInput").ap(),
        "Wv": nc.dram_tensor("Wv", [C, C], f32, kind="ExternalInput").ap(),
        "Wo_r": nc.dram_tensor("Wo_r", [128, 4, C], f32,
                               kind="ExternalInput").ap(),
        "ones_row": nc.dram_tensor("ones_row", [1, 128], f32,
                                   kind="ExternalInput").ap(),
        "ones_v": nc.dram_tensor("ones_v", [128, HEADS, HEADS, 1], f32,
                                 kind="ExternalInput").ap(),
    }
    outs = {
        "out": nc.dram_tensor("out", [FPC, N, C], f32, kind="ExternalOutput").ap(),
    }
    with tile.TileContext(nc) as tc:
        build_body(tc, outs, ins)
    nc.compile()
    _CACHED = nc
    return nc


def make_in_maps(hidden_states, F_mats, Wq, Wk, Wv, Wo):
    hs = np.ascontiguousarray(hidden_states, dtype=np.float32)
    F = np.array(F_mats, dtype=np.float32, copy=True)
    F[::FRAMES_PER_VIDEO] = PSEUDO_F          # first frame of each video
    coordsT = make_coordsT()
    Wo_r = np.ascontiguousarray(
        np.asarray(Wo, np.float32).reshape(4, 2, D, C).transpose(1, 2, 0, 3)
    ).reshape(128, 4, C)
    in_maps = []
    for c in range(NCORES):
        fr = slice(c * FPC, (c + 1) * FPC)
        # ft[y, f, x] = F[fr][f][x, y]
        ft = np.ascontiguousarray(np.einsum("fxy->yfx", F[fr]))
        in_maps.append({
            "ones_row": np.ones((1, 128), np.float32),
            "ones_v": np.ones((128, HEADS, HEADS, 1), np.float32),
            "hs": np.ascontiguousarray(hs[fr]),
            "ft": ft,
            "coordsT": coordsT,
            "Wq": np.asarray(Wq, np.float32),
            "Wk": np.asarray(Wk, np.float32),
            "Wv": np.asarray(Wv, np.float32),
            "Wo_r": Wo_r,
        })
    return in_maps


def kernel(hidden_states, F_mats, Wq, Wk, Wv, Wo):
    from concourse.bass_utils import run_bass_kernel_spmd

    nc = _build_program()
    in_maps = make_in_maps(hidden_states, F_mats, Wq, Wk, Wv, Wo)
    res = run_bass_kernel_spmd(nc, in_maps, core_ids=list(range(NCORES)))
    out = np.concatenate([res.results[c]["out"] for c in range(NCORES)], axis=0)
    return out.astype(np.float32)


if __name__ == "__main__":
    import jax

    rng = np.random.default_rng(0)
    fake = {
        "hidden_states": rng.standard_normal((B, N, C), dtype=np.float32),
        "F_mats": rng.standard_normal((B, 3, 3), dtype=np.float32),
        "Wq": rng.standard_normal((C, C), dtype=np.float32) * C ** -0.5,
        "Wk": rng.standard_normal((C, C), dtype=np.float32) * C ** -0.5,
        "Wv": rng.standard_normal((C, C), dtype=np.float32) * C ** -0.5,
        "Wo": rng.standard_normal((C, C), dtype=np.float32) * C ** -0.5,
    }
    out = kernel(**fake)
    print("out", out.shape, out.dtype, np.abs(out).mean())



# revision 4
# speedup vs baseline: 42.2284x; 42.2284x over previous
"""Trainium2 Bass kernel for EpiModule (epipolar sparse attention).

Full inputs -> full output. Shards B=32 frames data-parallel across 8
NeuronCores (4 frames per core); QKV/O weights replicated.

Per-core device program (all fp32 storage, float32r matmuls):
  per frame:
    lines   = F^T-block matmul against constant pixel coords     [3, 1024]
    cfcT    = |coords^T @ lines| * (1/||line||)  (scoresT layout) [1024m, 1024n]
    band    = (3/128) * max(cfc);  emask = min(exp(-decay*(cfc-band)), 1)
    hsT     = PE-transpose of hidden_states                       [512c, 1024n]
    qT,kT   = W^T @ hsT  ;  v65 = [hs @ Wv | ones]                (ones col -> denom)
    per head:
      scoresT = kT^T q  (PSUM), e = exp(scores/8), attn = min(emask,1)*e
      outT[65, n] = v65^T @ attn   (row 64 = softmax denominator)
      attn_norm = outT[0:64] * recip(denom)  (broadcast via ones-matmul)
    out = attn_norm^T @ Wo_r  -> DRAM

Softmax max-subtraction is skipped: scores = qk/8 + mask with mask <= 0 and
every query row has a mask entry >= -20 (verified on the generator
distribution), so exp() neither overflows nor denominator-underflows in fp32.
"""

import sys

if "/opt/trn_rl_repo" not in sys.path:
    sys.path.insert(0, "/opt/trn_rl_repo")

import numpy as np

B, N, C = 32, 1024, 512
HEADS, D = 8, 64
NCORES = 8
FPC = B // NCORES           # frames per core
F_MAT_SIZE = 256
PIXEL_BAND = 3.0
DECAY_ALPHA = 3.0
FRAMES_PER_VIDEO = 16

# pseudo "horizontal line" fundamental matrix: F~ @ [x,y,1] = [0,-1,y]
PSEUDO_F = np.array([[0.0, 0.0, 0.0],
                     [0.0, 0.0, -1.0],
                     [0.0, 1.0, 0.0]], dtype=np.float32)


def make_coordsT():
    feat = int(round(N ** 0.5))          # 32
    n = np.arange(N, dtype=np.float32)
    scale = F_MAT_SIZE / feat            # 8.0
    off = (scale - 1.0) / 2.0            # 3.5
    x = scale * (n % feat) + off
    y = scale * (n // feat) + off
    return np.stack([x, y, np.ones(N, np.float32)], axis=0)  # [3, N]


def build_body(tc, out_aps, in_aps, attn_bufs=6, e1_bufs=2):
    """Emit the per-core program. out_aps/in_aps: dicts of DRAM APs."""
    import contextlib

    import concourse.bass as bass  # noqa: F401
    from concourse import mybir
    from concourse.masks import make_identity

    nc = tc.nc
    f32 = mybir.dt.float32
    f32r = mybir.dt.float32r
    AF = mybir.ActivationFunctionType
    OP = mybir.AluOpType
    AX = mybir.AxisListType

    def mm(out, lhsT, rhs, **kw):
        nc.tensor.matmul(out, lhsT, rhs, **kw)

    hs_d = in_aps["hs"]                      # [4, 1024, 512]
    ft_d = in_aps["ft"]                      # [3, 4, 3]   FT[y,f,x] = F_f[x,y]
    ct_d = in_aps["coordsT"]                 # [3, 1024]
    wq_d = in_aps["Wq"].bitcast(f32r)        # [512, 512]
    wk_d = in_aps["Wk"].bitcast(f32r)
    wv_d = in_aps["Wv"].bitcast(f32r)
    wor_d = in_aps["Wo_r"].bitcast(f32r)     # [128, 4, 512]
    ones_r_d = in_aps["ones_row"].bitcast(f32r)   # [1, 128]
    ones_v_d = in_aps["ones_v"].bitcast(f32r)     # [128, 8, 8, 1]
    out_d = out_aps["out"]                   # [4, 1024, 512]

    ctx = contextlib.ExitStack()
    with ctx, nc.allow_low_precision(reason="float32r tiles are fp32-width"):
        consts = ctx.enter_context(tc.tile_pool(name="consts", bufs=1))
        big = ctx.enter_context(tc.tile_pool(name="big", bufs=1))
        attn_pool = ctx.enter_context(tc.tile_pool(name="attn", bufs=max(2, attn_bufs // 3)))
        e1_pool = ctx.enter_context(tc.tile_pool(name="e1", bufs=e1_bufs))
        hs_pool = ctx.enter_context(tc.tile_pool(name="hsld", bufs=2))
        rb_pool = ctx.enter_context(tc.tile_pool(name="rb", bufs=4))
        psA = ctx.enter_context(tc.tile_pool(name="psA", bufs=2, space="PSUM"))
        psB = ctx.enter_context(tc.tile_pool(name="psB", bufs=4, space="PSUM"))

        # ---------------- constants ----------------
        wq_sb = consts.tile([128, 4, 512], f32r)
        wk_sb = consts.tile([128, 4, 512], f32r)
        wv_sb = consts.tile([128, 4, 512], f32r)
        nc.sync.dma_start(out=wq_sb, in_=wq_d.rearrange("(cc p) d -> p cc d", p=128))
        nc.sync.dma_start(out=wk_sb, in_=wk_d.rearrange("(cc p) d -> p cc d", p=128))
        nc.sync.dma_start(out=wv_sb, in_=wv_d.rearrange("(cc p) d -> p cc d", p=128))
        wor_sb = consts.tile([128, 4, 512], f32r)
        nc.sync.dma_start(out=wor_sb, in_=wor_d)
        ct_sb = consts.tile([3, 1024], f32)
        nc.sync.dma_start(out=ct_sb, in_=ct_d)
        ft_sb = consts.tile([3, 4, 3], f32)
        nc.sync.dma_start(out=ft_sb, in_=ft_d)
        ident = consts.tile([128, 128], f32)
        make_identity(nc, ident)
        ones_row = consts.tile([1, 128], f32r)
        nc.sync.dma_start(out=ones_row, in_=ones_r_d)

        # persistent per-frame workspaces (serially reused across frames)
        hsT = big.tile([128, 4, 1024], f32r)      # [c%128, cc, n]
        qT = big.tile([128, 4, 1024], f32r)       # [d%128, dc, n]
        kT = big.tile([128, 4, 1024], f32r)
        v65 = big.tile([128, 8, 8, 65], f32r)     # [m%128, mt, h, d|one]
        cfc = big.tile([128, 8, 1024], f32)      # [m%128, mt, n]; becomes emask
        anorm = big.tile([128, 4, 1024], f32r)    # [(h%2)*64+d, h//2, n]
        lines = big.tile([3, 1024], f32)
        abn_sq = big.tile([2, 1024], f32)
        abn = big.tile([1, 1024], f32r)
        bandp = big.tile([128, 8], f32)
        band2 = big.tile([128, 1], f32)
        band1 = big.tile([1, 1], f32r)
        bandM = big.tile([128, 1], f32)
        band_col = big.tile([128, 1], f32)
        rec = big.tile([128, 1], f32)
        negdecay = big.tile([128, 1], f32)
        dxb = big.tile([128, 1], f32)

        nc.sync.dma_start(out=v65[:, :, :, 64:65], in_=ones_v_d)

        for f in range(FPC):
            # ============ mask construction ============
            ps_l = psA.tile([3, 1024], f32, tag="psA")
            for s in range(2):
                nc.tensor.matmul(ps_l[:, s * 512:(s + 1) * 512], ft_sb[:, f, :],
                                 ct_sb[:, s * 512:(s + 1) * 512],
                                 start=True, stop=True)
            nc.vector.tensor_copy(out=lines, in_=ps_l)

            nc.vector.tensor_tensor(out=abn_sq, in0=lines[0:2, :],
                                    in1=lines[0:2, :], op=OP.mult)
            nc.gpsimd.tensor_reduce(out=abn, in_=abn_sq, axis=AX.C, op=OP.add)
            nc.scalar.activation(out=abn, in_=abn, func=AF.Sqrt)
            nc.vector.tensor_scalar_add(out=abn, in0=abn, scalar1=1e-6)
            nc.vector.reciprocal(out=abn, in_=abn)

            ps_i = psA.tile([128, 1024], f32, tag="psA")
            for s in range(2):
                mm(ps_i[:, s * 512:(s + 1) * 512], ones_row,
                   abn[:, s * 512:(s + 1) * 512], start=True, stop=True)
            # lines *= 1/||line||  (rows of the broadcast are identical)
            nc.vector.tensor_tensor(out=lines, in0=lines, in1=ps_i[0:3, :],
                                    op=OP.mult)

            for mt in range(8):
                ps_c = psA.tile([128, 1024], f32, tag="psA")
                for s in range(2):
                    nc.tensor.matmul(ps_c[:, s * 512:(s + 1) * 512],
                                     ct_sb[:, mt * 128:(mt + 1) * 128],
                                     lines[:, s * 512:(s + 1) * 512],
                                     start=True, stop=True)
                # band partial from PSUM (fp32, fused abs)
                nc.vector.tensor_reduce(out=bandp[:, mt:mt + 1], in_=ps_c,
                                        axis=AX.X, op=OP.max,
                                        apply_absolute_value=True)
                # cfc = |raw| via sign-bit clear (lines pre-normalized)
                nc.vector.tensor_scalar(
                    out=cfc[:, mt, :].bitcast(mybir.dt.int32),
                    in0=ps_c.bitcast(mybir.dt.int32),
                    scalar1=0x7FFFFFFF, scalar2=None,
                    op0=OP.bitwise_and)

            nc.vector.tensor_reduce(out=band2, in_=bandp, axis=AX.X, op=OP.max)
            nc.gpsimd.tensor_reduce(out=band1, in_=band2, axis=AX.C, op=OP.max)
            ps_b = psB.tile([128, 512], f32, tag="psB")
            nc.tensor.matmul(ps_b[:, 0:1], ones_row.bitcast(f32),
                             band1.bitcast(f32), start=True, stop=True)
            nc.vector.tensor_copy(out=bandM, in_=ps_b[:, 0:1])
            nc.vector.tensor_scalar_mul(out=band_col, in0=bandM,
                                        scalar1=PIXEL_BAND / (F_MAT_SIZE // 2))
            nc.vector.tensor_scalar_add(out=rec, in0=band_col, scalar1=1e-6)
            nc.vector.reciprocal(out=rec, in_=rec)
            nc.vector.tensor_scalar_mul(out=negdecay, in0=rec, scalar1=-DECAY_ALPHA)
            nc.vector.scalar_tensor_tensor(out=dxb, in0=band_col,
                                           scalar=DECAY_ALPHA, in1=rec,
                                           op0=OP.mult, op1=OP.mult)
            # emask = min(exp(-decay*cfc + decay*band), 1), in place over cfc
            for mt in range(8):
                nc.scalar.activation(out=cfc[:, mt, :], in_=cfc[:, mt, :],
                                     func=AF.Exp, scale=negdecay, bias=dxb)
                nc.vector.tensor_scalar(out=cfc[:, mt, :], in0=cfc[:, mt, :],
                                        scalar1=1.0, scalar2=None, op0=OP.min)

            # ============ hs load + transpose ============
            for nt in range(8):
                hst = hs_pool.tile([128, 512], f32)
                nc.sync.dma_start(out=hst, in_=hs_d[f, nt * 128:(nt + 1) * 128, :])
                ps_t = psB.tile([128, 512], f32, tag="psB")
                for cc in range(4):
                    nc.tensor.transpose(ps_t[:, cc * 128:(cc + 1) * 128],
                                        hst[:, cc * 128:(cc + 1) * 128], ident)
                nc.vector.tensor_copy(
                    out=hsT[:, :, nt * 128:(nt + 1) * 128],
                    in_=ps_t.rearrange("p (cc n) -> p cc n", cc=4))

            # ============ q/k/v projections ============
            for dst, w_sb in ((qT, wq_sb), (kT, wk_sb)):
                for dc in range(4):
                    ps_q = psA.tile([128, 1024], f32, tag="psA")
                    for s in range(2):
                        for cc in range(4):
                            mm(ps_q[:, s * 512:(s + 1) * 512],
                               w_sb[:, cc, dc * 128:(dc + 1) * 128],
                               hsT[:, cc, s * 512:(s + 1) * 512],
                               start=(cc == 0), stop=(cc == 3))
                    nc.vector.tensor_copy(out=dst[:, dc, :], in_=ps_q)

            for nt in range(8):
                ps_v = psB.tile([128, 512], f32, tag="psB")
                for cc in range(4):
                    mm(ps_v, hsT[:, cc, nt * 128:(nt + 1) * 128], wv_sb[:, cc, :],
                       start=(cc == 0), stop=(cc == 3))
                nc.vector.tensor_copy(
                    out=v65[:, nt, :, 0:64],
                    in_=ps_v.rearrange("p (h d) -> p h d", h=8))

            # ============ attention ============
            for h in range(8):
                hb = (h % 2) * 64          # partition base of this head in qT/kT
                hc = h // 2                # chunk index
                ps_av0 = psB.tile([128, 512], f32, tag="psB")
                ps_av1 = psB.tile([128, 512], f32, tag="psB")
                ps_av = [ps_av0, ps_av1]
                for mp in range(4):          # mt pairs
                    e1 = e1_pool.tile([128, 2, 1024], f32)
                    for half in range(2):
                        mt = mp * 2 + half
                        ps_s = psA.tile([128, 1024], f32, tag="psA")
                        for s in range(2):
                            mm(ps_s[:, s * 512:(s + 1) * 512],
                               kT[hb:hb + 64, hc, mt * 128:(mt + 1) * 128],
                               qT[hb:hb + 64, hc, s * 512:(s + 1) * 512],
                               start=True, stop=True)
                        # exp to SBUF so the qk PSUM frees early
                        nc.scalar.activation(out=e1[:, half, :], in_=ps_s,
                                             func=AF.Exp, scale=1.0 / 8.0)
                    at = attn_pool.tile([128, 2, 1024], f32r)
                    if (h * 4 + mp) % 15 < 8:
                        nc.gpsimd.tensor_tensor(
                            out=at, in0=cfc[:, mp * 2:mp * 2 + 2, :],
                            in1=e1, op=OP.mult)
                    else:
                        nc.vector.tensor_tensor(
                            out=at, in0=cfc[:, mp * 2:mp * 2 + 2, :],
                            in1=e1, op=OP.mult)
                    for half in range(2):
                        mt = mp * 2 + half
                        for s in range(2):
                            mm(ps_av[s][0:65, :], v65[:, mt, h, :],
                               at[:, half, s * 512:(s + 1) * 512],
                               start=(mt == 0), stop=(mt == 7))

                for s in range(2):
                    rden = rb_pool.tile([1, 512], f32r, tag="rb")
                    nc.vector.reciprocal(out=rden, in_=ps_av[s][64:65, :])
                    ps_r = psB.tile([128, 512], f32, tag="psB")
                    mm(ps_r[0:64, :], ones_row[:, 0:64], rden,
                       start=True, stop=True)
                    rb = rb_pool.tile([64, 512], f32, tag="rb")
                    nc.vector.tensor_copy(out=rb, in_=ps_r[0:64, :])
                    if h % 2 == 0:
                        nc.vector.tensor_tensor(
                            out=anorm[0:64, h // 2, s * 512:(s + 1) * 512],
                            in0=ps_av[s][0:64, :], in1=rb, op=OP.mult)
                    else:
                        ntmp = rb_pool.tile([64, 512], f32r, tag="rb")
                        nc.vector.tensor_tensor(
                            out=ntmp, in0=ps_av[s][0:64, :], in1=rb, op=OP.mult)
                        nc.sync.dma_start(
                            out=anorm[64:128, h // 2, s * 512:(s + 1) * 512],
                            in_=ntmp)

            # ============ output projection ============
            for nt in range(8):
                ps_o = psB.tile([128, 512], f32, tag="psB")
                for j in range(4):
                    mm(ps_o, anorm[:, j, nt * 128:(nt + 1) * 128],
                       wor_sb[:, j, :],
                       start=(j == 0), stop=(j == 3))
                ot = rb_pool.tile([128, 512], f32, tag="rb")
                nc.scalar.copy(out=ot, in_=ps_o)
                nc.sync.dma_start(out=out_d[f, nt * 128:(nt + 1) * 128, :], in_=ot)


_CACHED = None


def _build_program():
    global _CACHED
    if _CACHED is not None:
        return _CACHED
    import concourse.tile as tile
    from concourse import bacc, mybir

    f32 = mybir.dt.float32
    nc = bacc.Bacc("TRN2", target_bir_lowering=False, debug=False,
                   num_devices=NCORES)
    ins = {
        "hs": nc.dram_tensor("hs", [FPC, N, C], f32, kind="ExternalInput").ap(),
        "ft": nc.dram_tensor("ft", [3, FPC, 3], f32, kind="ExternalInput").ap(),
        "coordsT": nc.dram_tensor("coordsT", [3, N], f32, kind="ExternalInput").ap(),
        "Wq": nc.dram_tensor("Wq", [C, C], f32, kind="ExternalInput").ap(),
        "Wk": nc.dram_tensor("Wk", [C, C], f32, kind="ExternalInput").ap(),
        "Wv": nc.dram_tensor("Wv", [C, C], f32, kind="ExternalInput").ap(),
        "Wo_r": nc.dram_tensor("Wo_r", [128, 4, C], f32,
                               kind="ExternalInput").ap(),
        "ones_row": nc.dram_tensor("ones_row", [1, 128], f32,
                                   kind="ExternalInput").ap(),
        "ones_v": nc.dram_tensor("ones_v", [128, HEADS, HEADS, 1], f32,
                                 kind="ExternalInput").ap(),
    }
    outs = {
        "out": nc.dram_tensor("out", [FPC, N, C], f32, kind="ExternalOutput").ap(),
    }
    with tile.TileContext(nc) as tc:
        build_body(tc, outs, ins)
    nc.compile()
    _CACHED = nc
    return nc


# ---------------------------------------------------------------------------
# Fast execution path: build the sharded PJRT executable ONCE and reuse it.
# Per-call inputs are passed as GLOBAL arrays (axis-0 sharded across the 8
# cores, or replicated); constants are staged on device at build time.
# ---------------------------------------------------------------------------

# which BIR inputs vary per kernel() call vs are constants of the problem
_VARIABLE_INS = ("hs", "ft", "Wq", "Wk", "Wv", "Wo_r")

_RUNNER = None


def _get_runner():
    global _RUNNER
    if _RUNNER is not None:
        return _RUNNER
    import jax
    from jax.sharding import Mesh, PartitionSpec, NamedSharding
    from jax.experimental.shard_map import shard_map
    from concourse import bass2jax as B2J
    from concourse import mybir

    nc = _build_program()
    B2J.install_neuronx_cc_hook()

    pname = nc.partition_id_tensor.name if nc.partition_id_tensor else None
    in_names, out_names, out_avals = [], [], []
    for alloc in nc.m.functions[0].allocations:
        if not isinstance(alloc, mybir.MemoryLocationSet):
            continue
        name = alloc.memorylocations[0].name
        if alloc.kind == "ExternalInput":
            if name != pname:
                in_names.append(name)
        elif alloc.kind == "ExternalOutput":
            out_names.append(name)
            out_avals.append(jax.core.ShapedArray(
                tuple(alloc.tensor_shape), mybir.dt.np(alloc.dtype)))
    all_in = in_names + out_names
    if pname is not None:
        all_in = all_in + [pname]

    def _body(*args):
        operands = list(args)
        if pname is not None:
            operands.append(B2J.partition_id_tensor())
        outs = B2J._bass_exec_p.bind(
            *operands, out_avals=tuple(out_avals), in_names=tuple(all_in),
            out_names=tuple(out_names), lowering_input_output_aliases=(),
            sim_require_finite=True, sim_require_nnan=True, nc=nc)
        return tuple(outs)

    devices = jax.devices()[:NCORES]
    mesh = Mesh(np.asarray(devices), ("core",))
    shard = NamedSharding(mesh, PartitionSpec("core"))
    repl = NamedSharding(mesh, PartitionSpec())

    # per-input partition spec: shard the per-frame tensors, replicate weights
    sharded_ins = {"hs", "ft"}
    in_specs = tuple(
        PartitionSpec("core") if nm in sharded_ins else PartitionSpec()
        for nm in in_names
    ) + (PartitionSpec("core"),) * len(out_names)
    out_specs = (PartitionSpec("core"),) * len(out_names)

    fn = jax.jit(
        shard_map(_body, mesh=mesh, in_specs=in_specs, out_specs=out_specs,
                  check_rep=False),
        keep_unused=True)

    # stage the call-invariant inputs + the (never-read) output seed buffer
    const_vals = {
        "coordsT": make_coordsT(),
        "ones_row": np.ones((1, 128), np.float32),
        "ones_v": np.ones((128, HEADS, HEADS, 1), np.float32),
    }
    staged = {}
    for nm in in_names:
        if nm in const_vals:
            staged[nm] = jax.device_put(const_vals[nm], repl)
    out_seed = [
        jax.device_put(
            np.zeros((NCORES * av.shape[0], *av.shape[1:]), av.dtype), shard)
        for av in out_avals
    ]

    def call(var_arrays):
        """var_arrays: dict name -> GLOBAL np/jax array for variable inputs."""
        args = []
        for nm in in_names:
            args.append(staged[nm] if nm in staged else var_arrays[nm])
        args.extend(out_seed)
        return fn(*args)

    _RUNNER = {
        "nc": nc, "fn": fn, "call": call, "mesh": mesh,
        "shard": shard, "repl": repl, "in_names": in_names,
        "staged": staged, "out_seed": out_seed, "sharded_ins": sharded_ins,
    }
    return _RUNNER


def host_prep(hidden_states, F_mats, Wq, Wk, Wv, Wo):
    """Host-side input massage -> dict of GLOBAL arrays for the runner."""
    F = np.array(F_mats, dtype=np.float32, copy=True)
    F[::FRAMES_PER_VIDEO] = PSEUDO_F
    # ftg: per-core [3, FPC, 3] with ft[y, f, x] = F[x, y]; concat on axis 0
    ftg = np.ascontiguousarray(
        np.einsum("cfxy->cyfx", F.reshape(NCORES, FPC, 3, 3))
    ).reshape(NCORES * 3, FPC, 3)
    Wo_r = np.ascontiguousarray(
        np.asarray(Wo, np.float32).reshape(4, 2, D, C).transpose(1, 2, 0, 3)
    ).reshape(128, 4, C)
    return {
        "hs": np.ascontiguousarray(hidden_states, dtype=np.float32),
        "ft": ftg,
        "Wq": np.asarray(Wq, np.float32),
        "Wk": np.asarray(Wk, np.float32),
        "Wv": np.asarray(Wv, np.float32),
        "Wo_r": Wo_r,
    }


def make_in_maps(hidden_states, F_mats, Wq, Wk, Wv, Wo):
    hs = np.ascontiguousarray(hidden_states, dtype=np.float32)
    F = np.array(F_mats, dtype=np.float32, copy=True)
    F[::FRAMES_PER_VIDEO] = PSEUDO_F          # first frame of each video
    coordsT = make_coordsT()
    Wo_r = np.ascontiguousarray(
        np.asarray(Wo, np.float32).reshape(4, 2, D, C).transpose(1, 2, 0, 3)
    ).reshape(128, 4, C)
    in_maps = []
    for c in range(NCORES):
        fr = slice(c * FPC, (c + 1) * FPC)
        # ft[y, f, x] = F[fr][f][x, y]
        ft = np.ascontiguousarray(np.einsum("fxy->yfx", F[fr]))
        in_maps.append({
            "ones_row": np.ones((1, 128), np.float32),
            "ones_v": np.ones((128, HEADS, HEADS, 1), np.float32),
            "hs": np.ascontiguousarray(hs[fr]),
            "ft": ft,
            "coordsT": coordsT,
            "Wq": np.asarray(Wq, np.float32),
            "Wk": np.asarray(Wk, np.float32),
            "Wv": np.asarray(Wv, np.float32),
            "Wo_r": Wo_r,
        })
    return in_maps


def kernel(hidden_states, F_mats, Wq, Wk, Wv, Wo):
    runner = _get_runner()
    var_arrays = host_prep(hidden_states, F_mats, Wq, Wk, Wv, Wo)
    out = runner["call"](var_arrays)[0]
    return np.asarray(out)


if __name__ == "__main__":
    import jax

    rng = np.random.default_rng(0)
    fake = {
        "hidden_states": rng.standard_normal((B, N, C), dtype=np.float32),
        "F_mats": rng.standard_normal((B, 3, 3), dtype=np.float32),
        "Wq": rng.standard_normal((C, C), dtype=np.float32) * C ** -0.5,
        "Wk": rng.standard_normal((C, C), dtype=np.float32) * C ** -0.5,
        "Wv": rng.standard_normal((C, C), dtype=np.float32) * C ** -0.5,
        "Wo": rng.standard_normal((C, C), dtype=np.float32) * C ** -0.5,
    }
    out = kernel(**fake)
    print("out", out.shape, out.dtype, np.abs(out).mean())



# revision 21
# speedup vs baseline: 45.1463x; 1.0691x over previous
"""Trainium2 Bass kernel for EpiModule (epipolar sparse attention).

Full inputs -> full output. Shards B=32 frames data-parallel across 8
NeuronCores (4 frames per core); QKV/O weights replicated.

Per-core device program (all fp32 storage, float32r matmuls):
  per frame:
    lines   = F^T-block matmul against constant pixel coords     [3, 1024]
    cfcT    = |coords^T @ lines| * (1/||line||)  (scoresT layout) [1024m, 1024n]
    band    = (3/128) * max(cfc);  emask = min(exp(-decay*(cfc-band)), 1)
    hsT     = PE-transpose of hidden_states                       [512c, 1024n]
    qT,kT   = W^T @ hsT  ;  v65 = [hs @ Wv | ones]                (ones col -> denom)
    per head:
      scoresT = kT^T q  (PSUM), e = exp(scores/8), attn = min(emask,1)*e
      outT[65, n] = v65^T @ attn   (row 64 = softmax denominator)
      attn_norm = outT[0:64] * recip(denom)  (broadcast via ones-matmul)
    out = attn_norm^T @ Wo_r  -> DRAM

Softmax max-subtraction is skipped: scores = qk/8 + mask with mask <= 0 and
every query row has a mask entry >= -20 (verified on the generator
distribution), so exp() neither overflows nor denominator-underflows in fp32.
"""

import sys

if "/opt/trn_rl_repo" not in sys.path:
    sys.path.insert(0, "/opt/trn_rl_repo")

import numpy as np

B, N, C = 32, 1024, 512
HEADS, D = 8, 64
NCORES = 8
FPC = B // NCORES           # frames per core
F_MAT_SIZE = 256
PIXEL_BAND = 3.0
DECAY_ALPHA = 3.0
FRAMES_PER_VIDEO = 16

# pseudo "horizontal line" fundamental matrix: F~ @ [x,y,1] = [0,-1,y]
PSEUDO_F = np.array([[0.0, 0.0, 0.0],
                     [0.0, 0.0, -1.0],
                     [0.0, 1.0, 0.0]], dtype=np.float32)


def make_coordsT():
    feat = int(round(N ** 0.5))          # 32
    n = np.arange(N, dtype=np.float32)
    scale = F_MAT_SIZE / feat            # 8.0
    off = (scale - 1.0) / 2.0            # 3.5
    x = scale * (n % feat) + off
    y = scale * (n // feat) + off
    return np.stack([x, y, np.ones(N, np.float32)], axis=0)  # [3, N]


def build_body(tc, out_aps, in_aps):
    """Emit the per-core program (software-pipelined across frames).

    Datapath: bf16 SBUF tiles for hsT/q/k/v/attention weights (PE rate is
    identical to f32r, DVE gets its 2x/4x fast modes, SBUF halves so the
    per-frame workspaces double-buffer); f32 PSUM accumulation throughout;
    the epipolar-distance tile (cfc) stays f32 because the band threshold
    is derived from its max. Frame f+1's hsT/qkv prep and mask matmuls are
    emitted interleaved into frame f's attention heads so the in-order PE
    queue always has independent work while the Activation engine runs the
    softmax exps (the critical resource: 8.4M exps/frame).
    """
    import contextlib

    import concourse.bass as bass  # noqa: F401
    from concourse import mybir
    from concourse.masks import make_identity
    from concourse import bass_isa

    nc = tc.nc
    f32 = mybir.dt.float32
    f32r = mybir.dt.float32r
    bf16 = mybir.dt.bfloat16
    AF = mybir.ActivationFunctionType
    OP = mybir.AluOpType
    AX = mybir.AxisListType

    def mm(out, lhsT, rhs, **kw):
        nc.tensor.matmul(out, lhsT, rhs, **kw)

    hs_d = in_aps["hs"]                      # [4, 1024, 512]
    ft_d = in_aps["ft"]                      # [3, 4, 3]   FT[y,f,x] = F_f[x,y]
    ct_d = in_aps["coordsT"]                 # [3, 1024]
    wq_d = in_aps["Wq"]                      # [512, 512]
    wk_d = in_aps["Wk"]
    wv_d = in_aps["Wv"]
    wor_d = in_aps["Wo_r"]                   # [128, 4, 512]
    out_d = out_aps["out"]                   # [4, 1024, 512]

    ctx = contextlib.ExitStack()
    with ctx, nc.allow_low_precision(reason="bf16 datapath, fp32 accumulate"):
        consts = ctx.enter_context(tc.tile_pool(name="consts", bufs=1))
        big = ctx.enter_context(tc.tile_pool(name="big", bufs=1))
        e1_pool = ctx.enter_context(tc.tile_pool(name="e1", bufs=2))
        at_pool = ctx.enter_context(tc.tile_pool(name="at", bufs=2))
        hs_pool = ctx.enter_context(tc.tile_pool(name="hsld", bufs=3))
        rb_pool = ctx.enter_context(tc.tile_pool(name="rb", bufs=4))
        ot_pool = ctx.enter_context(tc.tile_pool(name="ot", bufs=2))
        psA = ctx.enter_context(tc.tile_pool(name="psA", bufs=2, space="PSUM"))
        psB = ctx.enter_context(tc.tile_pool(name="psB", bufs=4, space="PSUM"))

        # ---------------- constants ----------------
        wq_sb = consts.tile([128, 4, 512], bf16)
        wk_sb = consts.tile([128, 4, 512], bf16)
        wv_sb = consts.tile([128, 4, 512], bf16)
        wor_sb = consts.tile([128, 4, 512], bf16)
        for w_sb, w_d in ((wq_sb, wq_d), (wk_sb, wk_d), (wv_sb, wv_d)):
            for cc in range(4):
                wst = hs_pool.tile([128, 512], f32, tag="hs", name="wst")
                nc.sync.dma_start(
                    out=wst, in_=w_d[cc * 128:(cc + 1) * 128, :])
                nc.vector.tensor_copy(out=w_sb[:, cc, :], in_=wst)
        for cc in range(4):
            wst = hs_pool.tile([128, 512], f32, tag="hs", name="wst")
            nc.sync.dma_start(out=wst, in_=wor_d[:, cc, :])
            nc.vector.tensor_copy(out=wor_sb[:, cc, :], in_=wst)
        ct_sb = consts.tile([3, 1024], f32)
        nc.sync.dma_start(out=ct_sb, in_=ct_d)
        ft_sb = consts.tile([3, 4, 3], f32)
        nc.sync.dma_start(out=ft_sb, in_=ft_d)
        ident = consts.tile([128, 128], f32)
        make_identity(nc, ident)

        # per-frame workspaces: the big tensors get two explicit sets so
        # frame f+1 prep overlaps frame f attention; the mask helpers are
        # transient (each frame's writes happen after the previous frame's
        # last read in program order) so one shared set suffices.
        dbl_specs = [
            ("hsT", [128, 4, 1024], bf16),    # [c%128, cc, n]
            ("qT", [128, 4, 1024], bf16),     # [d%128, dc, n]
            ("kT", [128, 4, 1024], bf16),
            ("v65", [128, 8, 8, 65], bf16),   # [m%128, mt, h, d|1]
            ("anorm", [128, 4, 1024], bf16),  # [(h%2)*64+d, h//2, n]
        ]
        shared_specs = [
            ("lines", [3, 1024], f32),
            ("abn2", [2, 1024], f32),
            ("rabn", [1, 1024], f32),
            ("rb3", [3, 1024], f32),
            ("bandp", [128, 8], f32),
            ("band2", [128, 1], f32),
            ("bandM", [128, 1], f32),
            ("band_col", [128, 1], f32),
            ("rec", [128, 1], f32),
            ("negdecay", [128, 1], f32),
            ("dxb", [128, 1], f32),
        ]
        shared = {nm: big.tile(shp, dt, name=nm)
                  for nm, shp, dt in shared_specs}
        frames = []
        for i in range(2):
            ws = {nm: big.tile(shp, dt, name=f"{nm}{i}")
                  for nm, shp, dt in dbl_specs}
            ws.update(shared)
            frames.append(ws)
        # single-buffered: raw distances live only between the mask matmuls
        # and the emask exps of the same frame
        cfc = big.tile([128, 8, 1024], f32)       # [m%128, mt, n]
        emask = big.tile([128, 8, 1024], bf16)    # min(exp(..), 1) per frame

        for fr in frames:
            nc.vector.memset(fr["v65"][:, :, :, 64:65], 1.0)

        # -------------- emission helpers --------------
        def emit_lines(f):
            """Epipolar lines + 1/||(a,b)|| normalization for frame f."""
            t = frames[f % 2]
            ps_l = psA.tile([3, 1024], f32, tag="psA")
            for s in range(2):
                mm(ps_l[:, s * 512:(s + 1) * 512], ft_sb[:, f, :],
                   ct_sb[:, s * 512:(s + 1) * 512], start=True, stop=True)
            nc.vector.tensor_copy(out=t["lines"], in_=ps_l)
            nc.vector.tensor_tensor(out=t["abn2"], in0=t["lines"][0:2, :],
                                    in1=t["lines"][0:2, :], op=OP.mult)
            nc.gpsimd.tensor_reduce(out=t["rabn"], in_=t["abn2"],
                                    axis=AX.C, op=OP.add)
            nc.scalar.activation(out=t["rabn"], in_=t["rabn"], func=AF.Sqrt)
            nc.vector.tensor_scalar_add(out=t["rabn"], in0=t["rabn"],
                                        scalar1=1e-6)
            nc.vector.reciprocal(out=t["rabn"], in_=t["rabn"])
            nc.gpsimd.partition_broadcast(t["rb3"], t["rabn"])
            nc.vector.tensor_tensor(out=t["lines"], in0=t["lines"],
                                    in1=t["rb3"], op=OP.mult)

        def emit_mask_chunk(f, mt):
            """|coords . line| tile mt + band partial for frame f."""
            t = frames[f % 2]
            ps_c = psA.tile([128, 1024], f32, tag="psA")
            for s in range(2):
                mm(ps_c[:, s * 512:(s + 1) * 512],
                   ct_sb[:, mt * 128:(mt + 1) * 128],
                   t["lines"][:, s * 512:(s + 1) * 512],
                   start=True, stop=True)
            # cfc = |raw| via sign-bit clear (lines pre-normalized)
            nc.vector.tensor_scalar(
                out=cfc[:, mt, :].bitcast(mybir.dt.int32),
                in0=ps_c.bitcast(mybir.dt.int32),
                scalar1=0x7FFFFFFF, scalar2=None, op0=OP.bitwise_and)
            nc.vector.tensor_reduce(out=t["bandp"][:, mt:mt + 1],
                                    in_=cfc[:, mt, :], axis=AX.X, op=OP.max)

        def emit_band_scalars(f):
            t = frames[f % 2]
            nc.vector.tensor_reduce(out=t["band2"], in_=t["bandp"],
                                    axis=AX.X, op=OP.max)
            nc.gpsimd.partition_all_reduce(
                t["bandM"], t["band2"], channels=128,
                reduce_op=bass_isa.ReduceOp.max)
            nc.vector.tensor_scalar_mul(
                out=t["band_col"], in0=t["bandM"],
                scalar1=PIXEL_BAND / (F_MAT_SIZE // 2))
            nc.vector.tensor_scalar_add(out=t["rec"], in0=t["band_col"],
                                        scalar1=1e-6)
            nc.vector.reciprocal(out=t["rec"], in_=t["rec"])
            nc.vector.tensor_scalar_mul(out=t["negdecay"], in0=t["rec"],
                                        scalar1=-DECAY_ALPHA)
            nc.vector.scalar_tensor_tensor(out=t["dxb"], in0=t["band_col"],
                                           scalar=DECAY_ALPHA, in1=t["rec"],
                                           op0=OP.mult, op1=OP.mult)

        def emit_emask(f, mt):
            """emask = min(exp(-decay*cfc + decay*band), 1) -> bf16."""
            t = frames[f % 2]
            nc.scalar.activation(out=emask[:, mt, :], in_=cfc[:, mt, :],
                                 func=AF.Exp, scale=t["negdecay"],
                                 bias=t["dxb"])
            nc.vector.tensor_scalar(out=emask[:, mt, :], in0=emask[:, mt, :],
                                    scalar1=1.0, scalar2=None, op0=OP.min)

        def emit_hst(f, nt):
            """Load + PE-transpose one 128-row chunk of hidden_states."""
            t = frames[f % 2]
            hst = hs_pool.tile([128, 512], f32, tag="hs", name="hst")
            nc.sync.dma_start(out=hst, in_=hs_d[f, nt * 128:(nt + 1) * 128, :])
            ps_t = psB.tile([128, 512], f32, tag="psB")
            for cc in range(4):
                nc.tensor.transpose(ps_t[:, cc * 128:(cc + 1) * 128],
                                    hst[:, cc * 128:(cc + 1) * 128], ident)
            nc.vector.tensor_copy(
                out=t["hsT"][:, :, nt * 128:(nt + 1) * 128],
                in_=ps_t.rearrange("p (cc n) -> p cc n", cc=4))

        def emit_qk(f, i):
            """One dc-chunk of the q (i<4) or k (i>=4) projection."""
            t = frames[f % 2]
            dst, w_sb = (t["qT"], wq_sb) if i < 4 else (t["kT"], wk_sb)
            dc = i % 4
            ps_q = psA.tile([128, 1024], f32, tag="psA")
            for s in range(2):
                for cc in range(4):
                    mm(ps_q[:, s * 512:(s + 1) * 512],
                       w_sb[:, cc, dc * 128:(dc + 1) * 128],
                       t["hsT"][:, cc, s * 512:(s + 1) * 512],
                       start=(cc == 0), stop=(cc == 3))
            nc.vector.tensor_copy(out=dst[:, dc, :], in_=ps_q)

        def emit_v(f, nt):
            t = frames[f % 2]
            ps_v = psB.tile([128, 512], f32, tag="psB")
            for cc in range(4):
                mm(ps_v, t["hsT"][:, cc, nt * 128:(nt + 1) * 128],
                   wv_sb[:, cc, :], start=(cc == 0), stop=(cc == 3))
            nc.vector.tensor_copy(
                out=t["v65"][:, nt, :, 0:64],
                in_=ps_v.rearrange("p (h d) -> p h d", h=8))

        def emit_attn_head(f, h):
            t = frames[f % 2]
            hb = (h % 2) * 64
            hc = h // 2
            ps_av = [psB.tile([128, 512], f32, tag="psB", name="ps_av0"),
                     psB.tile([128, 512], f32, tag="psB", name="ps_av1")]
            for mp in range(4):          # mt pairs
                e1 = e1_pool.tile([128, 2, 1024], bf16)
                for half in range(2):
                    mt = mp * 2 + half
                    ps_s = psA.tile([128, 1024], f32, tag="psA")
                    for s in range(2):
                        mm(ps_s[:, s * 512:(s + 1) * 512],
                           t["kT"][hb:hb + 64, hc, mt * 128:(mt + 1) * 128],
                           t["qT"][hb:hb + 64, hc, s * 512:(s + 1) * 512],
                           start=True, stop=True)
                    # exp to SBUF bf16 so the qk PSUM frees early
                    nc.scalar.activation(out=e1[:, half, :], in_=ps_s,
                                         func=AF.Exp, scale=1.0 / 8.0)
                at = at_pool.tile([128, 2, 1024], bf16)
                nc.vector.tensor_tensor(
                    out=at, in0=emask[:, mp * 2:mp * 2 + 2, :],
                    in1=e1, op=OP.mult)
                for half in range(2):
                    mt = mp * 2 + half
                    for s in range(2):
                        mm(ps_av[s][0:65, :], t["v65"][:, mt, h, :],
                           at[:, half, s * 512:(s + 1) * 512],
                           start=(mt == 0), stop=(mt == 7))

            for s in range(2):
                rden = rb_pool.tile([1, 512], f32, tag="rb", name="rden")
                nc.vector.reciprocal(out=rden, in_=ps_av[s][64:65, :])
                rb = rb_pool.tile([64, 512], f32, tag="rb")
                nc.gpsimd.partition_broadcast(rb, rden)
                if h % 2 == 0:
                    nc.vector.tensor_tensor(
                        out=t["anorm"][0:64, h // 2, s * 512:(s + 1) * 512],
                        in0=ps_av[s][0:64, :], in1=rb, op=OP.mult)
                else:
                    ntmp = rb_pool.tile([64, 512], bf16, tag="rb")
                    nc.vector.tensor_tensor(
                        out=ntmp, in0=ps_av[s][0:64, :], in1=rb, op=OP.mult)
                    nc.sync.dma_start(
                        out=t["anorm"][64:128, h // 2, s * 512:(s + 1) * 512],
                        in_=ntmp)

        def emit_out(f, nt):
            t = frames[f % 2]
            ps_o = psB.tile([128, 512], f32, tag="psB")
            for j in range(4):
                mm(ps_o, t["anorm"][:, j, nt * 128:(nt + 1) * 128],
                   wor_sb[:, j, :], start=(j == 0), stop=(j == 3))
            ot = ot_pool.tile([128, 512], f32)
            nc.vector.tensor_copy(out=ot, in_=ps_o)
            nc.sync.dma_start(out=out_d[f, nt * 128:(nt + 1) * 128, :], in_=ot)

        def emit_mask_tail(f):
            emit_band_scalars(f)
            for mt in range(8):
                emit_emask(f, mt)

        # -------------- software-pipelined schedule --------------
        # prologue: frame 0 fully prepped + masked (qk needs the full hsT,
        # so its chunks only start after the last hst chunk)
        emit_lines(0)
        for i in range(8):
            emit_hst(0, i)
            emit_v(0, i)
            emit_mask_chunk(0, i)
        for i in range(8):
            emit_qk(0, i)
        emit_mask_tail(0)

        for f in range(FPC):
            nxt = f + 1
            if nxt < FPC:
                emit_lines(nxt)
            # attention heads of frame f, interleaved with frame f+1 prep +
            # mask matmuls so the in-order PE queue never drains
            for h in range(8):
                emit_attn_head(f, h)
                if nxt < FPC:
                    emit_hst(nxt, h)
                    emit_v(nxt, h)
                    emit_mask_chunk(nxt, h)
            for nt in range(8):
                emit_out(f, nt)
            if nxt < FPC:
                for i in range(8):
                    emit_qk(nxt, i)
                emit_mask_tail(nxt)


_CACHED = None


def _build_program():
    global _CACHED
    if _CACHED is not None:
        return _CACHED
    import concourse.tile as tile
    from concourse import bacc, mybir

    f32 = mybir.dt.float32
    nc = bacc.Bacc("TRN2", target_bir_lowering=False, debug=False,
                   num_devices=NCORES)
    ins = {
        "hs": nc.dram_tensor("hs", [FPC, N, C], f32, kind="ExternalInput").ap(),
        "ft": nc.dram_tensor("ft", [3, FPC, 3], f32, kind="ExternalInput").ap(),
        "coordsT": nc.dram_tensor("coordsT", [3, N], f32, kind="ExternalInput").ap(),
        "Wq": nc.dram_tensor("Wq", [C, C], f32, kind="ExternalInput").ap(),
        "Wk": nc.dram_tensor("Wk", [C, C], f32, kind="ExternalInput").ap(),
        "Wv": nc.dram_tensor("Wv", [C, C], f32, kind="ExternalInput").ap(),
        "Wo_r": nc.dram_tensor("Wo_r", [128, 4, C], f32,
                               kind="ExternalInput").ap(),
    }
    outs = {
        "out": nc.dram_tensor("out", [FPC, N, C], f32, kind="ExternalOutput").ap(),
    }
    with tile.TileContext(nc) as tc:
        build_body(tc, outs, ins)
    nc.compile()
    _CACHED = nc
    return nc


# ---------------------------------------------------------------------------
# Fast execution path: build the sharded PJRT executable ONCE and reuse it.
# Per-call inputs are passed as GLOBAL arrays (axis-0 sharded across the 8
# cores, or replicated); constants are staged on device at build time.
# ---------------------------------------------------------------------------

# which BIR inputs vary per kernel() call vs are constants of the problem
_VARIABLE_INS = ("hs", "ft", "Wq", "Wk", "Wv", "Wo_r")

_RUNNER = None


def _get_runner():
    global _RUNNER
    if _RUNNER is not None:
        return _RUNNER
    import jax
    from jax.sharding import Mesh, PartitionSpec, NamedSharding
    from jax.experimental.shard_map import shard_map
    from concourse import bass2jax as B2J
    from concourse import mybir

    nc = _build_program()
    B2J.install_neuronx_cc_hook()

    pname = nc.partition_id_tensor.name if nc.partition_id_tensor else None
    in_names, out_names, out_avals = [], [], []
    for alloc in nc.m.functions[0].allocations:
        if not isinstance(alloc, mybir.MemoryLocationSet):
            continue
        name = alloc.memorylocations[0].name
        if alloc.kind == "ExternalInput":
            if name != pname:
                in_names.append(name)
        elif alloc.kind == "ExternalOutput":
            out_names.append(name)
            out_avals.append(jax.core.ShapedArray(
                tuple(alloc.tensor_shape), mybir.dt.np(alloc.dtype)))
    all_in = in_names + out_names
    if pname is not None:
        all_in = all_in + [pname]

    def _body(*args):
        operands = list(args)
        if pname is not None:
            operands.append(B2J.partition_id_tensor())
        outs = B2J._bass_exec_p.bind(
            *operands, out_avals=tuple(out_avals), in_names=tuple(all_in),
            out_names=tuple(out_names), lowering_input_output_aliases=(),
            sim_require_finite=True, sim_require_nnan=True, nc=nc)
        return tuple(outs)

    devices = jax.devices()[:NCORES]
    mesh = Mesh(np.asarray(devices), ("core",))
    shard = NamedSharding(mesh, PartitionSpec("core"))
    repl = NamedSharding(mesh, PartitionSpec())

    # per-input partition spec: shard the per-frame tensors, replicate weights
    sharded_ins = {"hs", "ft"}
    in_specs = tuple(
        PartitionSpec("core") if nm in sharded_ins else PartitionSpec()
        for nm in in_names
    ) + (PartitionSpec("core"),) * len(out_names)
    out_specs = (PartitionSpec("core"),) * len(out_names)

    fn = jax.jit(
        shard_map(_body, mesh=mesh, in_specs=in_specs, out_specs=out_specs,
                  check_rep=False),
        keep_unused=True)

    # stage the call-invariant inputs + the (never-read) output seed buffer
    const_vals = {
        "coordsT": make_coordsT(),
    }
    staged = {}
    for nm in in_names:
        if nm in const_vals:
            staged[nm] = jax.device_put(const_vals[nm], repl)
    out_seed = [
        jax.device_put(
            np.zeros((NCORES * av.shape[0], *av.shape[1:]), av.dtype), shard)
        for av in out_avals
    ]

    def call(var_arrays):
        """var_arrays: dict name -> GLOBAL np/jax array for variable inputs."""
        args = []
        for nm in in_names:
            args.append(staged[nm] if nm in staged else var_arrays[nm])
        args.extend(out_seed)
        return fn(*args)

    _RUNNER = {
        "nc": nc, "fn": fn, "call": call, "mesh": mesh,
        "shard": shard, "repl": repl, "in_names": in_names,
        "staged": staged, "out_seed": out_seed, "sharded_ins": sharded_ins,
    }
    return _RUNNER


def host_prep(hidden_states, F_mats, Wq, Wk, Wv, Wo):
    """Host-side input massage -> dict of GLOBAL arrays for the runner."""
    F = np.array(F_mats, dtype=np.float32, copy=True)
    F[::FRAMES_PER_VIDEO] = PSEUDO_F
    # ftg: per-core [3, FPC, 3] with ft[y, f, x] = F[x, y]; concat on axis 0
    ftg = np.ascontiguousarray(
        np.einsum("cfxy->cyfx", F.reshape(NCORES, FPC, 3, 3))
    ).reshape(NCORES * 3, FPC, 3)
    Wo_r = np.ascontiguousarray(
        np.asarray(Wo, np.float32).reshape(4, 2, D, C).transpose(1, 2, 0, 3)
    ).reshape(128, 4, C)
    return {
        "hs": np.ascontiguousarray(hidden_states, dtype=np.float32),
        "ft": ftg,
        "Wq": np.asarray(Wq, np.float32),
        "Wk": np.asarray(Wk, np.float32),
        "Wv": np.asarray(Wv, np.float32),
        "Wo_r": Wo_r,
    }


def make_in_maps(hidden_states, F_mats, Wq, Wk, Wv, Wo):
    hs = np.ascontiguousarray(hidden_states, dtype=np.float32)
    F = np.array(F_mats, dtype=np.float32, copy=True)
    F[::FRAMES_PER_VIDEO] = PSEUDO_F          # first frame of each video
    coordsT = make_coordsT()
    Wo_r = np.ascontiguousarray(
        np.asarray(Wo, np.float32).reshape(4, 2, D, C).transpose(1, 2, 0, 3)
    ).reshape(128, 4, C)
    in_maps = []
    for c in range(NCORES):
        fr = slice(c * FPC, (c + 1) * FPC)
        # ft[y, f, x] = F[fr][f][x, y]
        ft = np.ascontiguousarray(np.einsum("fxy->yfx", F[fr]))
        in_maps.append({
            "hs": np.ascontiguousarray(hs[fr]),
            "ft": ft,
            "coordsT": coordsT,
            "Wq": np.asarray(Wq, np.float32),
            "Wk": np.asarray(Wk, np.float32),
            "Wv": np.asarray(Wv, np.float32),
            "Wo_r": Wo_r,
        })
    return in_maps


def kernel(hidden_states, F_mats, Wq, Wk, Wv, Wo):
    runner = _get_runner()
    var_arrays = host_prep(hidden_states, F_mats, Wq, Wk, Wv, Wo)
    out = runner["call"](var_arrays)[0]
    return np.asarray(out)


if __name__ == "__main__":
    import jax

    rng = np.random.default_rng(0)
    fake = {
        "hidden_states": rng.standard_normal((B, N, C), dtype=np.float32),
        "F_mats": rng.standard_normal((B, 3, 3), dtype=np.float32),
        "Wq": rng.standard_normal((C, C), dtype=np.float32) * C ** -0.5,
        "Wk": rng.standard_normal((C, C), dtype=np.float32) * C ** -0.5,
        "Wv": rng.standard_normal((C, C), dtype=np.float32) * C ** -0.5,
        "Wo": rng.standard_normal((C, C), dtype=np.float32) * C ** -0.5,
    }
    out = kernel(**fake)
    print("out", out.shape, out.dtype, np.abs(out).mean())

